# revision 3
# baseline (speedup 1.0000x reference)
"""Cosine-similarity attention map on 8 Trainium2 NeuronCores.

out[b, i, j] = <x[b,:,i], x[b,:,j]> / (||x[b,:,i]|| * ||x[b,:,j]||)
x: [B=4, C=64, N=4096] fp32  ->  out: [B=4, N=4096, N=4096] fp32

The output is a symmetric Gram matrix, so each core computes only its
share of the (block) upper triangle, in fp16, and the host mirrors the
lower triangle while unsharding (rel tolerance is 2e-2; fp16 costs ~4e-4).

Sharding: 2 cores per batch. Global 128-row panels t = 0..31 of out[b];
core r in {0,1} owns panels t = 2p + r (p = 0..15 local). Panel t only
needs columns >= 128t; rounding down to 512-col chunks, local panel p
computes chunks floor(p/2)..7, i.e. width w = 8 - floor(p/2) in {8,8,7,7,
...,1,1} — identical for both cores, so one SPMD program serves all 8.
Row data comes from the same normalized tensor as column data (rows ==
cols of a Gram matrix): core r receives x[b] rolled left by 128*r
columns (pre-cast to fp16 on host), making lhsT = YF[:, 256p:256p+128]
for every core.

Engine budget per core (all ~25-30us): PE 88 matmuls (kept warm by a
dummy-matmul burst so the HAM clock gate reaches 2.4 GHz), PSUM
evacuation split DVE/ACT by projected finish time, squares on the
otherwise-idle GpSimd, DMA-out coalesced into >=1536-col pieces.
"""

import sys

sys.path.insert(0, "/opt/trn_rl_repo")

import numpy as np

import concourse.bass as bass
import concourse.mybir as mybir
import concourse.tile as tile
from concourse import bacc
from concourse.bass_utils import run_bass_kernel_spmd

B, C, N = 4, 64, 4096
NCORES = 8
RB = 2048  # 16 local 128-row panels per core
CH = 512  # norm / matmul column chunk
NCH = N // CH  # 8

F32 = mybir.dt.float32
F16 = mybir.dt.float16


def _build():
    nc = bacc.Bacc("TRN2", target_bir_lowering=False)
    xf = nc.declare_dram_parameter("xf", [C, N], F16, isOutput=False)
    out = nc.declare_dram_parameter("out", [RB, N], F16, isOutput=True)

    # Projected busy time (us) per copy engine; DVE at 0.96 GHz also does
    # the 8 chunk muls, ACT at 1.2 GHz also does the 8 abs_rsqrt.
    eng_t = {"v": 5.6, "a": 4.8}

    with tile.TileContext(nc) as tc:
        with (
            tc.tile_pool(name="persist", bufs=1) as persist,
            tc.tile_pool(name="panels", bufs=4) as panels,
            tc.tile_pool(name="mpsum", bufs=2, space="PSUM") as mpsum,
            tc.tile_pool(name="npsum", bufs=2, space="PSUM") as npsum,
            tc.tile_pool(name="wpsum", bufs=1, space="PSUM") as wpsum,
        ):
            # PE warm-up: ~10 dependency-free matmuls (~4.3us cold) flip the
            # HAM clock gate to 2.4 GHz before the real matmuls begin.
            GARB = persist.tile([C, CH], F16)
            nc.vector.memset(GARB, 0.5)
            WPS = wpsum.tile([128, CH], F32)
            for _ in range(10):
                nc.tensor.matmul(
                    WPS, lhsT=GARB[:, 0:128], rhs=GARB, start=True, stop=True
                )

            XF = persist.tile([C, N], F16)
            # Chunks are consumed descending (small panels first), so load
            # them in that order too, two chunks per DMA.
            for c in range(NCH - 2, -1, -2):
                cs = slice(c * CH, (c + 2) * CH)
                nc.sync.dma_start(out=XF[:, cs], in_=xf[:, cs])

            ones_f = persist.tile([C, 1], F32)
            nc.vector.memset(ones_f, 1.0)
            ones_c = persist.tile([C, 1], F16)  # sumsq reduction lhsT
            nc.vector.tensor_copy(ones_c, ones_f)
            ones_rf = persist.tile([1, C], F32)
            nc.vector.memset(ones_rf, 1.0)
            ones_r = persist.tile([1, C], F16)  # K=1 partition-broadcast lhsT
            nc.vector.tensor_copy(ones_r, ones_rf)

            SQ = persist.tile([C, N], F16)
            RN16 = persist.tile([1, N], F16)
            YF = persist.tile([C, N], F16)

            # x^2 on the otherwise-idle GpSimd, descending, as loads land.
            for c in range(NCH - 1, -1, -1):
                cs = slice(c * CH, (c + 1) * CH)
                nc.gpsimd.tensor_mul(SQ[:, cs], XF[:, cs], XF[:, cs])

            # Normalize columns of one 512-col chunk: y = x * rsqrt(sumsq).
            def norm_chunk(c):
                cs = slice(c * CH, (c + 1) * CH)
                pps = npsum.tile([128, CH], F32, tag="pps")
                nc.tensor.matmul(
                    pps[0:1, :], lhsT=ones_c, rhs=SQ[:, cs], start=True, stop=True
                )
                nc.scalar.activation(
                    RN16[:, cs],
                    pps[0:1, :],
                    mybir.ActivationFunctionType.Abs_reciprocal_sqrt,
                )
                nc.tensor.matmul(
                    pps[0:C, :], lhsT=ones_r, rhs=RN16[:, cs], start=True, stop=True
                )
                nc.vector.tensor_mul(YF[:, cs], XF[:, cs], pps[0:C, :])

            def copy_balanced(dst, src, cols):
                if eng_t["v"] <= eng_t["a"]:
                    eng_t["v"] += 0.130 + cols / 960.0
                    nc.vector.tensor_copy(dst, src)
                else:
                    eng_t["a"] += 0.145 + cols / 1200.0
                    nc.scalar.copy(out=dst, in_=src)

            # Panels 2c and 2c+1: rhs chunks c..7, lhsT inside chunk c.
            def emit_panels(c):
                js = list(range(c, NCH))
                groups = []
                i = len(js) % 2
                if i:
                    groups.append(js[:1])
                while i < len(js):
                    groups.append(js[i : i + 2])
                    i += 2
                for p in (2 * c, 2 * c + 1):
                    pnl = panels.tile([128, N], F16, tag="panel")
                    rs_ = slice(128 * p, 128 * (p + 1))
                    lhsT = YF[:, 256 * p : 256 * p + 128]
                    flush0 = 0  # start col (local) of un-DMA'd span
                    pend = 0
                    for gi, g in enumerate(groups):
                        ps = mpsum.tile([128, 2 * CH], F32, tag="ps")
                        for qi, j in enumerate(g):
                            nc.tensor.matmul(
                                ps[:, qi * CH : (qi + 1) * CH],
                                lhsT=lhsT,
                                rhs=YF[:, j * CH : (j + 1) * CH],
                                start=True,
                                stop=True,
                            )
                        lc = slice((g[0] - c) * CH, (g[0] - c + len(g)) * CH)
                        copy_balanced(pnl[:, lc], ps[:, : len(g) * CH], len(g) * CH)
                        pend += len(g) * CH
                        if pend >= 3 * CH or gi == len(groups) - 1:
                            fl = slice(flush0, flush0 + pend)
                            nc.sync.dma_start(out=out[rs_, fl], in_=pnl[:, fl])
                            flush0 += pend
                            pend = 0

            # Software-pipelined: panels for chunk c are emitted after the
            # norm of chunk c-1 so the norm chain never queues behind the
            # bulk matmul/copy work on DVE/ACT/PE.
            norm_chunk(NCH - 1)
            for c in range(NCH - 2, -1, -1):
                norm_chunk(c)
                emit_panels(c + 1)
            emit_panels(0)

    nc.compile()
    return nc


def _install_profile_hook():
    """This container's antenv lacks axon_hooks, so run_bass_kernel_spmd's
    trace=True path dies on import. Recreate the module and register the
    ctypes NTFF hook that trn_boot would have installed."""
    import sys as _sys
    import types

    if "antenv.axon_hooks" in _sys.modules:
        return
    import antenv

    mod = types.ModuleType("antenv.axon_hooks")
    mod._hook = None

    def set_axon_ntff_profile_hook(h):
        mod._hook = h

    def get_axon_ntff_profile_hook():
        return mod._hook

    mod.set_axon_ntff_profile_hook = set_axon_ntff_profile_hook
    mod.get_axon_ntff_profile_hook = get_axon_ntff_profile_hook
    _sys.modules["antenv.axon_hooks"] = mod
    antenv.axon_hooks = mod

    from trn_agent_boot.trn_boot import _ntff_profile_via_ctypes

    mod.set_axon_ntff_profile_hook(
        _ntff_profile_via_ctypes("/opt/axon/libaxon_pjrt.so")
    )


_nc = None


def _get_nc():
    global _nc
    if _nc is None:
        _nc = _build()
    return _nc


def _run(x, trace=False, trace_cores=None):
    x = np.asarray(x, dtype=np.float32)
    assert x.shape == (B, C, N), x.shape
    core_ids = list(range(NCORES))
    in_maps = []
    for k in core_ids:
        b, r = divmod(k, 2)
        xb = x[b] if r == 0 else np.roll(x[b], -128, axis=1)
        in_maps.append({"xf": np.ascontiguousarray(xb.astype(np.float16))})
    if trace:
        _install_profile_hook()
    res = run_bass_kernel_spmd(
        _get_nc(), in_maps, core_ids, trace=trace, trace_cores=trace_cores
    )
    out = np.empty((B, N, N), dtype=np.float32)
    for k in core_ids:
        b, r = divmod(k, 2)
        S = res.results[k]["out"]  # [2048, 4096] fp16
        for p in range(16):
            t = 2 * p + r
            ss = 512 * (p // 2)  # chunk-aligned col start (shifted coords)
            L = (N - ss) - 128 * r  # valid slab length (clip wraparound)
            cs = ss + 128 * r  # actual col start
            out[b, 128 * t : 128 * (t + 1), cs : cs + L] = S[
                128 * p : 128 * (p + 1), 0:L
            ]
    # Mirror the block lower triangle from the computed upper part.
    for b in range(B):
        ob = out[b]
        for t in range(1, 32):
            fs = 512 * (t // 4) + 128 * (t % 2)
            if fs:
                ob[128 * t : 128 * (t + 1), 0:fs] = ob[
                    0:fs, 128 * t : 128 * (t + 1)
                ].T
    return out, res


def kernel(x):
    return _run(x)[0]


# revision 4
# speedup vs baseline: 1.3277x; 1.3277x over previous
"""Cosine-similarity attention map on 8 Trainium2 NeuronCores.

out[b, i, j] = <x[b,:,i], x[b,:,j]> / (||x[b,:,i]|| * ||x[b,:,j]||)
x: [B=4, C=64, N=4096] fp32  ->  out: [B=4, N=4096, N=4096] fp32

The output is a symmetric Gram matrix, so each core computes only its
share of the (block) upper triangle, in fp16, and the host mirrors the
lower triangle while unsharding (rel tolerance is 2e-2; fp16 costs ~4e-4).

Sharding: 2 cores per batch. Global 128-row panels t = 0..31 of out[b];
core r in {0,1} owns panels t = 2p + r (p = 0..15 local). Panel t only
needs columns >= 128t; rounding down to 512-col chunks, local panel p
computes chunks floor(p/2)..7, i.e. width w = 8 - floor(p/2) in {8,8,7,7,
...,1,1} — identical for both cores, so one SPMD program serves all 8.
Row data comes from the same normalized tensor as column data (rows ==
cols of a Gram matrix): core r receives x[b] rolled left by 128*r
columns (pre-cast to fp16 on host), making lhsT = YF[:, 256p:256p+128]
for every core.

Engine budget per core (all ~25-30us): PE 88 matmuls (kept warm by a
dummy-matmul burst so the HAM clock gate reaches 2.4 GHz), PSUM
evacuation split DVE/ACT by projected finish time, squares on the
otherwise-idle GpSimd, DMA-out coalesced into >=1536-col pieces.
"""

import sys

sys.path.insert(0, "/opt/trn_rl_repo")

import numpy as np

import concourse.bass as bass
import concourse.mybir as mybir
import concourse.tile as tile
from concourse import bacc
from concourse.bass_utils import run_bass_kernel_spmd

B, C, N = 4, 64, 4096
NCORES = 8
RB = 2048  # 16 local 128-row panels per core
CH = 512  # norm / matmul column chunk
NCH = N // CH  # 8

F32 = mybir.dt.float32
F16 = mybir.dt.float16


def _build():
    nc = bacc.Bacc("TRN2", target_bir_lowering=False)
    xf = nc.declare_dram_parameter("xf", [2 * C, N], F16, isOutput=False)
    out = nc.declare_dram_parameter("out", [RB, N], F16, isOutput=True)

    # Projected busy time (us) per copy engine; DVE at 0.96 GHz also does
    # the 8 chunk muls, ACT at 1.2 GHz also does the 8 abs_rsqrt.
    eng_t = {"v": 5.6, "a": 4.8}

    with tile.TileContext(nc) as tc:
        with (
            tc.tile_pool(name="persist", bufs=1) as persist,
            tc.tile_pool(name="panels", bufs=4) as panels,
            tc.tile_pool(name="mpsum", bufs=2, space="PSUM") as mpsum,
            tc.tile_pool(name="npsum", bufs=2, space="PSUM") as npsum,
            tc.tile_pool(name="wpsum", bufs=1, space="PSUM") as wpsum,
        ):
            # PE warm-up: ~10 dependency-free matmuls (~4.3us cold) flip the
            # HAM clock gate to 2.4 GHz before the real matmuls begin.
            GARB = persist.tile([2 * C, CH], F16)
            nc.vector.memset(GARB, 0.5)
            WPS = wpsum.tile([128, CH], F32)
            for _ in range(10):
                nc.tensor.matmul(
                    WPS, lhsT=GARB[:, 0:128], rhs=GARB, start=True, stop=True
                )

            XF = persist.tile([2 * C, N], F16)
            # Chunks are consumed descending (small panels first), so load
            # them in that order too, two chunks per DMA.
            for c in range(NCH - 2, -1, -2):
                cs = slice(c * CH, (c + 2) * CH)
                nc.sync.dma_start(out=XF[:, cs], in_=xf[:, cs])

            ones_f = persist.tile([2 * C, 1], F32)
            nc.vector.memset(ones_f, 1.0)
            ones_c = persist.tile([2 * C, 1], F16)  # sumsq reduction lhsT
            nc.vector.tensor_copy(ones_c, ones_f)
            ones_rf = persist.tile([1, 2 * C], F32)
            nc.vector.memset(ones_rf, 1.0)
            ones_r = persist.tile([1, 2 * C], F16)  # K=1 partition-broadcast lhsT
            nc.vector.tensor_copy(ones_r, ones_rf)

            SQ = persist.tile([2 * C, N], F16)
            RN16 = persist.tile([1, N], F16)
            YF = persist.tile([2 * C, N], F16)

            # x^2 on the otherwise-idle GpSimd, descending, as loads land.
            for c in range(NCH - 1, -1, -1):
                cs = slice(c * CH, (c + 1) * CH)
                nc.gpsimd.tensor_mul(SQ[:, cs], XF[:, cs], XF[:, cs])

            # Normalize columns of one 512-col chunk: y = x * rsqrt(sumsq).
            def norm_chunk(c):
                cs = slice(c * CH, (c + 1) * CH)
                pps = npsum.tile([128, CH], F32, tag="pps")
                nc.tensor.matmul(
                    pps[0:1, :], lhsT=ones_c, rhs=SQ[:, cs], start=True, stop=True
                )
                nc.scalar.activation(
                    RN16[:, cs],
                    pps[0:1, :],
                    mybir.ActivationFunctionType.Abs_reciprocal_sqrt,
                )
                nc.tensor.matmul(
                    pps[0 : 2 * C, :], lhsT=ones_r, rhs=RN16[:, cs], start=True, stop=True
                )
                nc.vector.tensor_mul(YF[:, cs], XF[:, cs], pps[0 : 2 * C, :])

            def copy_balanced(dst, src, cols):
                if eng_t["v"] <= eng_t["a"]:
                    eng_t["v"] += 0.130 + cols / 960.0
                    nc.vector.tensor_copy(dst, src)
                else:
                    eng_t["a"] += 0.145 + cols / 1200.0
                    nc.scalar.copy(out=dst, in_=src)

            # Panels 2c and 2c+1: rhs chunks c..7, lhsT inside chunk c.
            def emit_panels(c):
                js = list(range(c, NCH))
                groups = []
                i = len(js) % 2
                if i:
                    groups.append(js[:1])
                while i < len(js):
                    groups.append(js[i : i + 2])
                    i += 2
                for p in (2 * c, 2 * c + 1):
                    pnl = panels.tile([128, N], F16, tag="panel")
                    rs_ = slice(128 * p, 128 * (p + 1))
                    lhsT = YF[:, 256 * p : 256 * p + 128]
                    flush0 = 0  # start col (local) of un-DMA'd span
                    pend = 0
                    for gi, g in enumerate(groups):
                        ps = mpsum.tile([128, 2 * CH], F32, tag="ps")
                        for qi, j in enumerate(g):
                            nc.tensor.matmul(
                                ps[:, qi * CH : (qi + 1) * CH],
                                lhsT=lhsT,
                                rhs=YF[:, j * CH : (j + 1) * CH],
                                start=True,
                                stop=True,
                            )
                        lc = slice((g[0] - c) * CH, (g[0] - c + len(g)) * CH)
                        copy_balanced(pnl[:, lc], ps[:, : len(g) * CH], len(g) * CH)
                        pend += len(g) * CH
                        if pend >= 3 * CH or gi == len(groups) - 1:
                            fl = slice(flush0, flush0 + pend)
                            nc.sync.dma_start(out=out[rs_, fl], in_=pnl[:, fl])
                            flush0 += pend
                            pend = 0

            # Software-pipelined: panels for chunk c are emitted after the
            # norm of chunk c-1 so the norm chain never queues behind the
            # bulk matmul/copy work on DVE/ACT/PE.
            norm_chunk(NCH - 1)
            for c in range(NCH - 2, -1, -1):
                norm_chunk(c)
                emit_panels(c + 1)
            emit_panels(0)

    nc.compile()
    return nc


def _install_profile_hook():
    """This container's antenv lacks axon_hooks, so run_bass_kernel_spmd's
    trace=True path dies on import. Recreate the module and register the
    ctypes NTFF hook that trn_boot would have installed."""
    import sys as _sys
    import types

    if "antenv.axon_hooks" in _sys.modules:
        return
    import antenv

    mod = types.ModuleType("antenv.axon_hooks")
    mod._hook = None

    def set_axon_ntff_profile_hook(h):
        mod._hook = h

    def get_axon_ntff_profile_hook():
        return mod._hook

    mod.set_axon_ntff_profile_hook = set_axon_ntff_profile_hook
    mod.get_axon_ntff_profile_hook = get_axon_ntff_profile_hook
    _sys.modules["antenv.axon_hooks"] = mod
    antenv.axon_hooks = mod

    from trn_agent_boot.trn_boot import _ntff_profile_via_ctypes

    mod.set_axon_ntff_profile_hook(
        _ntff_profile_via_ctypes("/opt/axon/libaxon_pjrt.so")
    )


_nc = None


def _get_nc():
    global _nc
    if _nc is None:
        _nc = _build()
    return _nc


def _run(x, trace=False, trace_cores=None):
    x = np.asarray(x, dtype=np.float32)
    assert x.shape == (B, C, N), x.shape
    core_ids = list(range(NCORES))
    in_maps = []
    for k in core_ids:
        b, r = divmod(k, 2)
        xb = x[b] if r == 0 else np.roll(x[b], -128, axis=1)
        xb16 = xb.astype(np.float16)
        in_maps.append({"xf": np.ascontiguousarray(np.vstack([xb16, xb16]))})
    if trace:
        _install_profile_hook()
    res = run_bass_kernel_spmd(
        _get_nc(), in_maps, core_ids, trace=trace, trace_cores=trace_cores
    )
    out = np.empty((B, N, N), dtype=np.float32)
    for k in core_ids:
        b, r = divmod(k, 2)
        S = res.results[k]["out"]  # [2048, 4096] fp16
        for p in range(16):
            t = 2 * p + r
            ss = 512 * (p // 2)  # chunk-aligned col start (shifted coords)
            L = (N - ss) - 128 * r  # valid slab length (clip wraparound)
            cs = ss + 128 * r  # actual col start
            out[b, 128 * t : 128 * (t + 1), cs : cs + L] = S[
                128 * p : 128 * (p + 1), 0:L
            ]
    # Mirror the block lower triangle from the computed upper part.
    for b in range(B):
        ob = out[b]
        for t in range(1, 32):
            fs = 512 * (t // 4) + 128 * (t % 2)
            if fs:
                ob[128 * t : 128 * (t + 1), 0:fs] = ob[
                    0:fs, 128 * t : 128 * (t + 1)
                ].T
    return out, res


def kernel(x):
    return _run(x)[0]


# revision 6
# speedup vs baseline: 1.3613x; 1.0253x over previous
"""Cosine-similarity attention map on 8 Trainium2 NeuronCores.

out[b, i, j] = <x[b,:,i], x[b,:,j]> / (||x[b,:,i]|| * ||x[b,:,j]||)
x: [B=4, C=64, N=4096] fp32  ->  out: [B=4, N=4096, N=4096] fp32

The output is a symmetric Gram matrix, so each core computes only its
share of the (block) upper triangle, in fp16, and the host mirrors the
lower triangle while unsharding (rel tolerance is 2e-2; fp16 costs ~4e-4).

Sharding: 2 cores per batch. Global 128-row panels t = 0..31 of out[b];
core r in {0,1} owns panels t = 2p + r (p = 0..15 local). Panel t only
needs columns >= 128t; rounding down to 512-col chunks, local panel p
computes chunks floor(p/2)..7, i.e. width w = 8 - floor(p/2) in {8,8,7,7,
...,1,1} — identical for both cores, so one SPMD program serves all 8.
Row data comes from the same normalized tensor as column data (rows ==
cols of a Gram matrix): core r receives x[b] rolled left by 128*r
columns (pre-cast to fp16 on host), making lhsT = YF[:, 256p:256p+128]
for every core.

Engine budget per core (all ~25-30us): PE 88 matmuls (kept warm by a
dummy-matmul burst so the HAM clock gate reaches 2.4 GHz), PSUM
evacuation split DVE/ACT by projected finish time, squares on the
otherwise-idle GpSimd, DMA-out coalesced into >=1536-col pieces.
"""

import sys

sys.path.insert(0, "/opt/trn_rl_repo")

import numpy as np

import concourse.bass as bass
import concourse.mybir as mybir
import concourse.tile as tile
from concourse import bacc
from concourse.bass_utils import run_bass_kernel_spmd

B, C, N = 4, 64, 4096
NCORES = 8
RB = 2048  # 16 local 128-row panels per core
CH = 512  # norm / matmul column chunk
NCH = N // CH  # 8

F32 = mybir.dt.float32
F16 = mybir.dt.float16


def _build():
    nc = bacc.Bacc("TRN2", target_bir_lowering=False)
    xf = nc.declare_dram_parameter("xf", [2 * C, N], F16, isOutput=False)
    out = nc.declare_dram_parameter("out", [RB, N], F16, isOutput=True)

    # Projected busy time (us) per copy engine; DVE at 0.96 GHz also does
    # the 8 chunk muls, ACT at 1.2 GHz also does the 8 abs_rsqrt.
    eng_t = {"v": 5.5, "a": 6.0}

    with tile.TileContext(nc) as tc:
        with (
            tc.tile_pool(name="persist", bufs=1) as persist,
            tc.tile_pool(name="panels", bufs=6) as panels,
            tc.tile_pool(name="mpsum", bufs=3, space="PSUM") as mpsum,
            tc.tile_pool(name="npsum", bufs=2, space="PSUM") as npsum,
        ):
            # PE warm-up: 7 dependency-free matmuls (~4us cold) flip the
            # HAM clock gate to 2.4 GHz before the real matmuls begin.
            GARB = persist.tile([2 * C, CH], F16)
            nc.vector.memset(GARB, 0.5)
            WPS = npsum.tile([128, CH], F32, tag="pps")
            for _ in range(7):
                nc.tensor.matmul(
                    WPS, lhsT=GARB[:, 0:128], rhs=GARB, start=True, stop=True
                )

            XF = persist.tile([2 * C, N], F16)
            # Chunks are consumed descending (small panels first), so load
            # them in that order too, two chunks per DMA.
            for c in range(NCH - 2, -1, -2):
                cs = slice(c * CH, (c + 2) * CH)
                nc.sync.dma_start(out=XF[:, cs], in_=xf[:, cs])

            ones_f = persist.tile([2 * C, 1], F32)
            nc.vector.memset(ones_f, 1.0)
            ones_c = persist.tile([2 * C, 1], F16)  # sumsq reduction lhsT
            nc.vector.tensor_copy(ones_c, ones_f)
            ones_rf = persist.tile([1, 2 * C], F32)
            nc.vector.memset(ones_rf, 1.0)
            ones_r = persist.tile([1, 2 * C], F16)  # K=1 partition-broadcast lhsT
            nc.vector.tensor_copy(ones_r, ones_rf)

            SQ = persist.tile([2 * C, N], F16)
            RN16 = persist.tile([1, N], F16)
            YF = persist.tile([2 * C, N], F16)

            # x^2 on the otherwise-idle GpSimd, descending, as loads land.
            for c in range(NCH - 1, -1, -1):
                cs = slice(c * CH, (c + 1) * CH)
                nc.gpsimd.tensor_mul(SQ[:, cs], XF[:, cs], XF[:, cs])

            # Normalize columns of one 512-col chunk: y = x * rsqrt(sumsq).
            def norm_chunk(c):
                cs = slice(c * CH, (c + 1) * CH)
                pps = npsum.tile([128, CH], F32, tag="pps")
                nc.tensor.matmul(
                    pps[0:1, :], lhsT=ones_c, rhs=SQ[:, cs], start=True, stop=True
                )
                nc.scalar.activation(
                    RN16[:, cs],
                    pps[0:1, :],
                    mybir.ActivationFunctionType.Abs_reciprocal_sqrt,
                )
                nc.tensor.matmul(
                    pps[0 : 2 * C, :], lhsT=ones_r, rhs=RN16[:, cs], start=True, stop=True
                )
                nc.vector.tensor_mul(YF[:, cs], XF[:, cs], pps[0 : 2 * C, :])

            def copy_balanced(dst, src, cols):
                if eng_t["v"] <= eng_t["a"]:
                    eng_t["v"] += 0.100 + cols / 1010.0
                    nc.vector.tensor_copy(dst, src)
                else:
                    eng_t["a"] += 0.120 + cols / 1020.0
                    nc.scalar.copy(out=dst, in_=src)

            # Panels 2c and 2c+1: rhs chunks c..7, lhsT inside chunk c.
            def emit_panels(c):
                js = list(range(c, NCH))
                groups = []
                i = len(js) % 2
                if i:
                    groups.append(js[:1])
                while i < len(js):
                    groups.append(js[i : i + 2])
                    i += 2
                for p in (2 * c, 2 * c + 1):
                    pnl = panels.tile([128, N], F16, tag="panel")
                    rs_ = slice(128 * p, 128 * (p + 1))
                    lhsT = YF[:, 256 * p : 256 * p + 128]
                    flush0 = 0  # start col (local) of un-DMA'd span
                    pend = 0
                    for gi, g in enumerate(groups):
                        ps = mpsum.tile([128, 2 * CH], F32, tag="ps")
                        for qi, j in enumerate(g):
                            nc.tensor.matmul(
                                ps[:, qi * CH : (qi + 1) * CH],
                                lhsT=lhsT,
                                rhs=YF[:, j * CH : (j + 1) * CH],
                                start=True,
                                stop=True,
                            )
                        lc = slice((g[0] - c) * CH, (g[0] - c + len(g)) * CH)
                        copy_balanced(pnl[:, lc], ps[:, : len(g) * CH], len(g) * CH)
                        pend += len(g) * CH
                        if pend >= 3 * CH or gi == len(groups) - 1:
                            fl = slice(flush0, flush0 + pend)
                            nc.sync.dma_start(out=out[rs_, fl], in_=pnl[:, fl])
                            flush0 += pend
                            pend = 0

            # Software-pipelined: panels for chunk c are emitted after the
            # norm of chunk c-1 so the norm chain never queues behind the
            # bulk matmul/copy work on DVE/ACT/PE.
            norm_chunk(NCH - 1)
            for c in range(NCH - 2, -1, -1):
                norm_chunk(c)
                emit_panels(c + 1)
            emit_panels(0)

    nc.compile()
    return nc


def _install_profile_hook():
    """This container's antenv lacks axon_hooks, so run_bass_kernel_spmd's
    trace=True path dies on import. Recreate the module and register the
    ctypes NTFF hook that trn_boot would have installed."""
    import sys as _sys
    import types

    if "antenv.axon_hooks" in _sys.modules:
        return
    import antenv

    mod = types.ModuleType("antenv.axon_hooks")
    mod._hook = None

    def set_axon_ntff_profile_hook(h):
        mod._hook = h

    def get_axon_ntff_profile_hook():
        return mod._hook

    mod.set_axon_ntff_profile_hook = set_axon_ntff_profile_hook
    mod.get_axon_ntff_profile_hook = get_axon_ntff_profile_hook
    _sys.modules["antenv.axon_hooks"] = mod
    antenv.axon_hooks = mod

    from trn_agent_boot.trn_boot import _ntff_profile_via_ctypes

    mod.set_axon_ntff_profile_hook(
        _ntff_profile_via_ctypes("/opt/axon/libaxon_pjrt.so")
    )


_nc = None


def _get_nc():
    global _nc
    if _nc is None:
        _nc = _build()
    return _nc


def _run(x, trace=False, trace_cores=None):
    x = np.asarray(x, dtype=np.float32)
    assert x.shape == (B, C, N), x.shape
    core_ids = list(range(NCORES))
    in_maps = []
    for k in core_ids:
        b, r = divmod(k, 2)
        xb = x[b] if r == 0 else np.roll(x[b], -128, axis=1)
        xb16 = xb.astype(np.float16)
        in_maps.append({"xf": np.ascontiguousarray(np.vstack([xb16, xb16]))})
    if trace:
        _install_profile_hook()
    res = run_bass_kernel_spmd(
        _get_nc(), in_maps, core_ids, trace=trace, trace_cores=trace_cores
    )
    out = np.empty((B, N, N), dtype=np.float32)
    for k in core_ids:
        b, r = divmod(k, 2)
        S = res.results[k]["out"]  # [2048, 4096] fp16
        for p in range(16):
            t = 2 * p + r
            ss = 512 * (p // 2)  # chunk-aligned col start (shifted coords)
            L = (N - ss) - 128 * r  # valid slab length (clip wraparound)
            cs = ss + 128 * r  # actual col start
            out[b, 128 * t : 128 * (t + 1), cs : cs + L] = S[
                128 * p : 128 * (p + 1), 0:L
            ]
    # Mirror the block lower triangle from the computed upper part.
    for b in range(B):
        ob = out[b]
        for t in range(1, 32):
            fs = 512 * (t // 4) + 128 * (t % 2)
            if fs:
                ob[128 * t : 128 * (t + 1), 0:fs] = ob[
                    0:fs, 128 * t : 128 * (t + 1)
                ].T
    return out, res


def kernel(x):
    return _run(x)[0]


# revision 7
# speedup vs baseline: 1.3781x; 1.0123x over previous
"""Cosine-similarity attention map on 8 Trainium2 NeuronCores.

out[b, i, j] = <x[b,:,i], x[b,:,j]> / (||x[b,:,i]|| * ||x[b,:,j]||)
x: [B=4, C=64, N=4096] fp32  ->  out: [B=4, N=4096, N=4096] fp32

The output is a symmetric Gram matrix, so each core computes only its
share of the (block) upper triangle, in fp16, and the host mirrors the
lower triangle while unsharding (rel tolerance is 2e-2; fp16 costs ~4e-4).

Sharding: 2 cores per batch. Global 128-row panels t = 0..31 of out[b];
core r in {0,1} owns panels t = 2p + r (p = 0..15 local). Panel t only
needs columns >= 128t; rounding down to 512-col chunks, local panel p
computes chunks floor(p/2)..7, i.e. width w = 8 - floor(p/2) in {8,8,7,7,
...,1,1} — identical for both cores, so one SPMD program serves all 8.
Row data comes from the same normalized tensor as column data (rows ==
cols of a Gram matrix): core r receives x[b] rolled left by 128*r
columns (pre-cast to fp16 on host), making lhsT = YF[:, 256p:256p+128]
for every core.

Engine budget per core (all ~25-30us): PE 88 matmuls (kept warm by a
dummy-matmul burst so the HAM clock gate reaches 2.4 GHz), PSUM
evacuation split DVE/ACT by projected finish time, squares on the
otherwise-idle GpSimd, DMA-out coalesced into >=1536-col pieces.
"""

import sys

sys.path.insert(0, "/opt/trn_rl_repo")

import numpy as np

import concourse.bass as bass
import concourse.mybir as mybir
import concourse.tile as tile
from concourse import bacc
from concourse.bass_utils import run_bass_kernel_spmd

B, C, N = 4, 64, 4096
NCORES = 8
RB = 2048  # 16 local 128-row panels per core
CH = 512  # norm / matmul column chunk
NCH = N // CH  # 8

F32 = mybir.dt.float32
F16 = mybir.dt.float16


def _build():
    nc = bacc.Bacc("TRN2", target_bir_lowering=False)
    xf = nc.declare_dram_parameter("xf", [2 * C, N], F16, isOutput=False)
    out = nc.declare_dram_parameter("out", [RB, N], F16, isOutput=True)

    # Projected busy time (us) per copy engine; DVE at 0.96 GHz also does
    # the 8 chunk muls, ACT at 1.2 GHz also does the 8 abs_rsqrt.
    eng_t = {"v": 5.5, "a": 6.0}

    with tile.TileContext(nc) as tc:
        with (
            tc.tile_pool(name="persist", bufs=1) as persist,
            tc.tile_pool(name="panels", bufs=6) as panels,
            tc.tile_pool(name="mpsum", bufs=2, space="PSUM") as mpsum,
            tc.tile_pool(name="wpsum", bufs=1, space="PSUM") as wpsum,
            tc.tile_pool(name="npsum", bufs=2, space="PSUM") as npsum,
        ):
            # PE warm-up: 12 dependency-free matmuls (~7us cold, two full HAM
            # windows) flip the clock gate to 2.4 GHz before the real matmuls.
            GARB = persist.tile([2 * C, CH], F16)
            nc.vector.memset(GARB, 0.5)
            WPS = wpsum.tile([128, CH], F32)
            for _ in range(12):
                nc.tensor.matmul(
                    WPS, lhsT=GARB[:, 0:128], rhs=GARB, start=True, stop=True
                )

            XF = persist.tile([2 * C, N], F16)
            # Chunks are consumed descending (small panels first), so load
            # them in that order too, two chunks per DMA.
            for c in range(NCH - 2, -1, -2):
                cs = slice(c * CH, (c + 2) * CH)
                nc.sync.dma_start(out=XF[:, cs], in_=xf[:, cs])

            ones_f = persist.tile([2 * C, 1], F32)
            nc.vector.memset(ones_f, 1.0)
            ones_c = persist.tile([2 * C, 1], F16)  # sumsq reduction lhsT
            nc.vector.tensor_copy(ones_c, ones_f)
            ones_rf = persist.tile([1, 2 * C], F32)
            nc.vector.memset(ones_rf, 1.0)
            ones_r = persist.tile([1, 2 * C], F16)  # K=1 partition-broadcast lhsT
            nc.vector.tensor_copy(ones_r, ones_rf)

            SQ = persist.tile([2 * C, N], F16)
            RN16 = persist.tile([1, N], F16)
            YF = persist.tile([2 * C, N], F16)

            # x^2 on the otherwise-idle GpSimd, descending, as loads land.
            for c in range(NCH - 1, -1, -1):
                cs = slice(c * CH, (c + 1) * CH)
                nc.gpsimd.tensor_mul(SQ[:, cs], XF[:, cs], XF[:, cs])

            # Normalize columns of one 512-col chunk: y = x * rsqrt(sumsq).
            def norm_chunk(c):
                cs = slice(c * CH, (c + 1) * CH)
                pps = npsum.tile([128, CH], F32, tag="pps")
                nc.tensor.matmul(
                    pps[0:1, :], lhsT=ones_c, rhs=SQ[:, cs], start=True, stop=True
                )
                nc.scalar.activation(
                    RN16[:, cs],
                    pps[0:1, :],
                    mybir.ActivationFunctionType.Abs_reciprocal_sqrt,
                )
                nc.tensor.matmul(
                    pps[0 : 2 * C, :], lhsT=ones_r, rhs=RN16[:, cs], start=True, stop=True
                )
                nc.vector.tensor_mul(YF[:, cs], XF[:, cs], pps[0 : 2 * C, :])

            def copy_balanced(dst, src, cols):
                if eng_t["v"] <= eng_t["a"]:
                    eng_t["v"] += 0.100 + cols / 1010.0
                    nc.vector.tensor_copy(dst, src)
                else:
                    eng_t["a"] += 0.120 + cols / 1020.0
                    nc.scalar.copy(out=dst, in_=src)

            # Panels 2c and 2c+1: rhs chunks c..7, lhsT inside chunk c.
            def emit_panels(c):
                js = list(range(c, NCH))
                groups = []
                i = len(js) % 2
                if i:
                    groups.append(js[:1])
                while i < len(js):
                    groups.append(js[i : i + 2])
                    i += 2
                for p in (2 * c, 2 * c + 1):
                    pnl = panels.tile([128, N], F16, tag="panel")
                    rs_ = slice(128 * p, 128 * (p + 1))
                    lhsT = YF[:, 256 * p : 256 * p + 128]
                    flush0 = 0  # start col (local) of un-DMA'd span
                    pend = 0
                    for gi, g in enumerate(groups):
                        ps = mpsum.tile([128, 2 * CH], F32, tag="ps")
                        for qi, j in enumerate(g):
                            nc.tensor.matmul(
                                ps[:, qi * CH : (qi + 1) * CH],
                                lhsT=lhsT,
                                rhs=YF[:, j * CH : (j + 1) * CH],
                                start=True,
                                stop=True,
                            )
                        lc = slice((g[0] - c) * CH, (g[0] - c + len(g)) * CH)
                        copy_balanced(pnl[:, lc], ps[:, : len(g) * CH], len(g) * CH)
                        pend += len(g) * CH
                        if pend >= 3 * CH or gi == len(groups) - 1:
                            fl = slice(flush0, flush0 + pend)
                            nc.sync.dma_start(out=out[rs_, fl], in_=pnl[:, fl])
                            flush0 += pend
                            pend = 0

            # Dependency-free matmuls queued at stage boundaries run exactly
            # when the PE would otherwise idle, keeping the HAM busy-fraction
            # high so the 2.4 GHz clock survives the dependency gaps.
            def warm_fill(n):
                for _ in range(n):
                    nc.tensor.matmul(
                        WPS, lhsT=GARB[:, 0:128], rhs=GARB, start=True, stop=True
                    )

            # Software-pipelined: panels for chunk c are emitted after the
            # norm of chunk c-1 so the norm chain never queues behind the
            # bulk matmul/copy work on DVE/ACT/PE.
            norm_chunk(NCH - 1)
            for c in range(NCH - 2, -1, -1):
                norm_chunk(c)
                warm_fill(2)
                emit_panels(c + 1)
                if c >= 2:
                    warm_fill(2)
            emit_panels(0)

    nc.compile()
    return nc


def _install_profile_hook():
    """This container's antenv lacks axon_hooks, so run_bass_kernel_spmd's
    trace=True path dies on import. Recreate the module and register the
    ctypes NTFF hook that trn_boot would have installed."""
    import sys as _sys
    import types

    if "antenv.axon_hooks" in _sys.modules:
        return
    import antenv

    mod = types.ModuleType("antenv.axon_hooks")
    mod._hook = None

    def set_axon_ntff_profile_hook(h):
        mod._hook = h

    def get_axon_ntff_profile_hook():
        return mod._hook

    mod.set_axon_ntff_profile_hook = set_axon_ntff_profile_hook
    mod.get_axon_ntff_profile_hook = get_axon_ntff_profile_hook
    _sys.modules["antenv.axon_hooks"] = mod
    antenv.axon_hooks = mod

    from trn_agent_boot.trn_boot import _ntff_profile_via_ctypes

    mod.set_axon_ntff_profile_hook(
        _ntff_profile_via_ctypes("/opt/axon/libaxon_pjrt.so")
    )


_nc = None


def _get_nc():
    global _nc
    if _nc is None:
        _nc = _build()
    return _nc


def _run(x, trace=False, trace_cores=None):
    x = np.asarray(x, dtype=np.float32)
    assert x.shape == (B, C, N), x.shape
    core_ids = list(range(NCORES))
    in_maps = []
    for k in core_ids:
        b, r = divmod(k, 2)
        xb = x[b] if r == 0 else np.roll(x[b], -128, axis=1)
        xb16 = xb.astype(np.float16)
        in_maps.append({"xf": np.ascontiguousarray(np.vstack([xb16, xb16]))})
    if trace:
        _install_profile_hook()
    res = run_bass_kernel_spmd(
        _get_nc(), in_maps, core_ids, trace=trace, trace_cores=trace_cores
    )
    out = np.empty((B, N, N), dtype=np.float32)
    for k in core_ids:
        b, r = divmod(k, 2)
        S = res.results[k]["out"]  # [2048, 4096] fp16
        for p in range(16):
            t = 2 * p + r
            ss = 512 * (p // 2)  # chunk-aligned col start (shifted coords)
            L = (N - ss) - 128 * r  # valid slab length (clip wraparound)
            cs = ss + 128 * r  # actual col start
            out[b, 128 * t : 128 * (t + 1), cs : cs + L] = S[
                128 * p : 128 * (p + 1), 0:L
            ]
    # Mirror the block lower triangle from the computed upper part.
    for b in range(B):
        ob = out[b]
        for t in range(1, 32):
            fs = 512 * (t // 4) + 128 * (t % 2)
            if fs:
                ob[128 * t : 128 * (t + 1), 0:fs] = ob[
                    0:fs, 128 * t : 128 * (t + 1)
                ].T
    return out, res


def kernel(x):
    return _run(x)[0]


# revision 9
# speedup vs baseline: 1.6068x; 1.1660x over previous
"""Cosine-similarity attention map on 8 Trainium2 NeuronCores.

out[b, i, j] = <x[b,:,i], x[b,:,j]> / (||x[b,:,i]|| * ||x[b,:,j]||)
x: [B=4, C=64, N=4096] fp32  ->  out: [B=4, N=4096, N=4096] fp32

The output is a symmetric Gram matrix, so each core computes only its
share of the (block) upper triangle, in fp16, and the host mirrors the
lower triangle while unsharding (rel tolerance is 2e-2; fp16 costs ~4e-4).

Sharding: 2 cores per batch. Global 128-row panels t = 0..31 of out[b];
core r in {0,1} owns panels t = 2p + r (p = 0..15 local). Panel t only
needs columns >= 128t; rounding down to 512-col chunks, local panel p
computes chunks floor(p/2)..7, i.e. width w = 8 - floor(p/2) in {8,8,7,7,
...,1,1} — identical for both cores, so one SPMD program serves all 8.
Row data comes from the same normalized tensor as column data (rows ==
cols of a Gram matrix): core r receives x[b] rolled left by 128*r
columns (pre-cast to fp16 on host), making lhsT = YF[:, 256p:256p+128]
for every core.

Engine budget per core (all ~25-30us): PE 88 matmuls (kept warm by a
dummy-matmul burst so the HAM clock gate reaches 2.4 GHz), PSUM
evacuation split DVE/ACT by projected finish time, squares on the
otherwise-idle GpSimd, DMA-out coalesced into >=1536-col pieces.
"""

import sys

sys.path.insert(0, "/opt/trn_rl_repo")

import numpy as np

import concourse.bass as bass
import concourse.mybir as mybir
import concourse.tile as tile
from concourse import bacc
from concourse.bass_utils import run_bass_kernel_spmd

B, C, N = 4, 64, 4096
NCORES = 8
RB = 2048  # 16 local 128-row panels per core
CH = 512  # norm / matmul column chunk
NCH = N // CH  # 8

F32 = mybir.dt.float32
F16 = mybir.dt.float16


def _build():
    nc = bacc.Bacc("TRN2", target_bir_lowering=False)
    xf = nc.declare_dram_parameter("xf", [2 * C, N], F16, isOutput=False)
    out = nc.declare_dram_parameter("out", [RB, N], F16, isOutput=True)

    # Projected busy time (us) per copy engine; DVE at 0.96 GHz also does
    # the 8 chunk muls, ACT at 1.2 GHz also does the 8 abs_rsqrt.
    eng_t = {"v": 5.5, "a": 6.0}

    with tile.TileContext(nc) as tc:
        with (
            tc.tile_pool(name="persist", bufs=1) as persist,
            tc.tile_pool(name="panels", bufs=6) as panels,
            tc.tile_pool(name="mpsum", bufs=3, space="PSUM") as mpsum,
            tc.tile_pool(name="npsum", bufs=2, space="PSUM") as npsum,
        ):
            # PE warm-up: 12 dependency-free matmuls (~7us cold, two full HAM
            # windows) flip the clock gate to 2.4 GHz before the real matmuls.
            GARB = persist.tile([2 * C, CH], F16)
            nc.vector.memset(GARB, 0.5)
            WPS = mpsum.tile([128, 2 * CH], F32, tag="ps")
            for _ in range(12):
                nc.tensor.matmul(
                    WPS[:, 0:CH], lhsT=GARB[:, 0:128], rhs=GARB, start=True, stop=True
                )

            XF = persist.tile([2 * C, N], F16)
            # Chunks are consumed descending (small panels first), so load
            # them in that order too, two chunks per DMA.
            for c in range(NCH - 2, -1, -2):
                cs = slice(c * CH, (c + 2) * CH)
                nc.sync.dma_start(out=XF[:, cs], in_=xf[:, cs])

            ones_f = persist.tile([2 * C, 1], F32)
            nc.vector.memset(ones_f, 1.0)
            ones_c = persist.tile([2 * C, 1], F16)  # sumsq reduction lhsT
            nc.vector.tensor_copy(ones_c, ones_f)
            ones_rf = persist.tile([1, 2 * C], F32)
            nc.vector.memset(ones_rf, 1.0)
            ones_r = persist.tile([1, 2 * C], F16)  # K=1 partition-broadcast lhsT
            nc.vector.tensor_copy(ones_r, ones_rf)

            SQ = persist.tile([2 * C, N], F16)
            RN16 = persist.tile([1, N], F16)
            YF = persist.tile([2 * C, N], F16)

            # x^2 on the otherwise-idle GpSimd, descending, as loads land.
            for c in range(NCH - 1, -1, -1):
                cs = slice(c * CH, (c + 1) * CH)
                nc.gpsimd.tensor_mul(SQ[:, cs], XF[:, cs], XF[:, cs])

            # Normalize columns of one 512-col chunk: y = x * rsqrt(sumsq).
            def norm_chunk(c):
                cs = slice(c * CH, (c + 1) * CH)
                pps = npsum.tile([128, CH], F32, tag="pps")
                nc.tensor.matmul(
                    pps[0:1, :], lhsT=ones_c, rhs=SQ[:, cs], start=True, stop=True
                )
                nc.scalar.activation(
                    RN16[:, cs],
                    pps[0:1, :],
                    mybir.ActivationFunctionType.Abs_reciprocal_sqrt,
                )
                nc.tensor.matmul(
                    pps[0 : 2 * C, :], lhsT=ones_r, rhs=RN16[:, cs], start=True, stop=True
                )
                nc.vector.tensor_mul(YF[:, cs], XF[:, cs], pps[0 : 2 * C, :])

            def copy_balanced(dst, src, cols):
                if eng_t["v"] <= eng_t["a"]:
                    eng_t["v"] += 0.100 + cols / 1010.0
                    nc.vector.tensor_copy(dst, src)
                else:
                    eng_t["a"] += 0.120 + cols / 1020.0
                    nc.scalar.copy(out=dst, in_=src)

            # Panels 2c and 2c+1: rhs chunks c..7, lhsT inside chunk c.
            def emit_panels(c):
                js = list(range(c, NCH))
                groups = []
                i = len(js) % 2
                if i:
                    groups.append(js[:1])
                while i < len(js):
                    groups.append(js[i : i + 2])
                    i += 2
                for p in (2 * c, 2 * c + 1):
                    pnl = panels.tile([128, N], F16, tag="panel")
                    rs_ = slice(128 * p, 128 * (p + 1))
                    lhsT = YF[:, 256 * p : 256 * p + 128]
                    flush0 = 0  # start col (local) of un-DMA'd span
                    pend = 0
                    for gi, g in enumerate(groups):
                        ps = mpsum.tile([128, 2 * CH], F32, tag="ps")
                        for qi, j in enumerate(g):
                            nc.tensor.matmul(
                                ps[:, qi * CH : (qi + 1) * CH],
                                lhsT=lhsT,
                                rhs=YF[:, j * CH : (j + 1) * CH],
                                start=True,
                                stop=True,
                            )
                        lc = slice((g[0] - c) * CH, (g[0] - c + len(g)) * CH)
                        copy_balanced(pnl[:, lc], ps[:, : len(g) * CH], len(g) * CH)
                        pend += len(g) * CH
                        if pend >= 3 * CH or gi == len(groups) - 1:
                            fl = slice(flush0, flush0 + pend)
                            nc.sync.dma_start(out=out[rs_, fl], in_=pnl[:, fl])
                            flush0 += pend
                            pend = 0

            # Software-pipelined: panels for chunk c are emitted after the
            # norm of chunk c-1 so the norm chain never queues behind the
            # bulk matmul/copy work on DVE/ACT/PE.
            norm_chunk(NCH - 1)
            for c in range(NCH - 2, -1, -1):
                norm_chunk(c)
                emit_panels(c + 1)
            emit_panels(0)

    nc.compile()
    return nc


def _install_profile_hook():
    """This container's antenv lacks axon_hooks, so run_bass_kernel_spmd's
    trace=True path dies on import. Recreate the module and register the
    ctypes NTFF hook that trn_boot would have installed."""
    import sys as _sys
    import types

    if "antenv.axon_hooks" in _sys.modules:
        return
    import antenv

    mod = types.ModuleType("antenv.axon_hooks")
    mod._hook = None

    def set_axon_ntff_profile_hook(h):
        mod._hook = h

    def get_axon_ntff_profile_hook():
        return mod._hook

    mod.set_axon_ntff_profile_hook = set_axon_ntff_profile_hook
    mod.get_axon_ntff_profile_hook = get_axon_ntff_profile_hook
    _sys.modules["antenv.axon_hooks"] = mod
    antenv.axon_hooks = mod

    from trn_agent_boot.trn_boot import _ntff_profile_via_ctypes

    mod.set_axon_ntff_profile_hook(
        _ntff_profile_via_ctypes("/opt/axon/libaxon_pjrt.so")
    )


_nc = None


def _get_nc():
    global _nc
    if _nc is None:
        _nc = _build()
    return _nc


def _run(x, trace=False, trace_cores=None):
    x = np.asarray(x, dtype=np.float32)
    assert x.shape == (B, C, N), x.shape
    core_ids = list(range(NCORES))
    in_maps = []
    for k in core_ids:
        b, r = divmod(k, 2)
        xb = x[b] if r == 0 else np.roll(x[b], -128, axis=1)
        xb16 = xb.astype(np.float16)
        in_maps.append({"xf": np.ascontiguousarray(np.vstack([xb16, xb16]))})
    if trace:
        _install_profile_hook()
    res = run_bass_kernel_spmd(
        _get_nc(), in_maps, core_ids, trace=trace, trace_cores=trace_cores
    )
    out = np.empty((B, N, N), dtype=np.float32)
    for k in core_ids:
        b, r = divmod(k, 2)
        S = res.results[k]["out"]  # [2048, 4096] fp16
        for p in range(16):
            t = 2 * p + r
            ss = 512 * (p // 2)  # chunk-aligned col start (shifted coords)
            L = (N - ss) - 128 * r  # valid slab length (clip wraparound)
            cs = ss + 128 * r  # actual col start
            out[b, 128 * t : 128 * (t + 1), cs : cs + L] = S[
                128 * p : 128 * (p + 1), 0:L
            ]
    # Mirror the block lower triangle from the computed upper part.
    for b in range(B):
        ob = out[b]
        for t in range(1, 32):
            fs = 512 * (t // 4) + 128 * (t % 2)
            if fs:
                ob[128 * t : 128 * (t + 1), 0:fs] = ob[
                    0:fs, 128 * t : 128 * (t + 1)
                ].T
    return out, res


def kernel(x):
    return _run(x)[0]
